# revision 16
# baseline (speedup 1.0000x reference)
"""Cached grouped-query multi-head attention on 8 Trainium2 cores.

Sharding: core c -> batch b = c//2, head-half = c%2 (8 of 16 heads, 2 of 4
KV groups per core). Wq/Wk column-parallel, Wo row-parallel; the two
partial Wo products per batch are summed on the host (the "all-reduce"),
which also adds bo.

All matmuls are bf16 (the PE moving-data port is 2 B/cycle/partition, so
fp8 DoubleRow only matches bf16 per contraction pair and any residual
compensation makes it slower; bare fp8 fails the 2e-2 gate).

Schedule notes:
- Inputs stream on three DMA queues; the rings share 16 DMA engines and
  saturate ~390 GB/s, so only the critical tensors (xt, wqa, wk) go in
  the first wave; wv/wqb/pkt/pv trail, wo is prefetched mid-attention.
- A short PE warm-up chain rides the initial DMA wait so the clock is at
  full p-state when real work lands.
- All projections share one psum pool by slicing the kps/qps tiles, so
  no pool-open barrier ever idles the PE between projection phases; both
  Q-proj halves run before attention (their ropes overlap attention PE).
- Attention per (g, qc, head-pair): per-head score matmuls into a
  [128,2,512] psum supertile -> one exp for both heads -> per-head PV.
  pv/den are emitted per kc-PAIR so the four M=32 denominator matmuls
  (head x kc-parity at PE columns 0/32/64/96) run concurrently, costing
  ~0.5 matmuls per key tile. The tail runs entirely on DVE+gpsimd (the
  ACT engine only ever runs exp, so scores never wait on it): psum
  drains, den = even+odd, one bf16 reciprocal (2x DVE rate) per pair of
  heads, and the normalize multiplies.
"""

import math
import sys

import numpy as np

sys.path.insert(0, "/opt/trn_rl_repo")

B, LQ, D = 4, 1024, 2048
H, G = 16, 4
HD = 128            # head dim
GS = H // G         # heads per group
PAST = 1024
LK = PAST + LQ      # 2048
NCORES = 8
NH = 8              # local heads per core
NG = 2              # local groups per core
KSUB = D // 128     # 16 contraction subtiles over D
QC = LQ // 512      # 2 query chunks of 512
QS = LQ // 128      # 8 query subtiles of 128
KC = LK // 128      # 16 key chunks of 128

_PERM = np.concatenate([np.arange(0, HD, 2), np.arange(1, HD, 2)])
_PROG_CACHE = {}


def _build_program(active):
    """active[qc] = [(kc, dcol, diag)]: dcol = first allowed query column
    (0 for full tiles); diag tiles have a triangular [128,128] block at
    query columns [dcol, dcol+128) and are fully allowed after it."""
    import concourse.bacc as bacc
    import concourse.mybir as mybir
    import concourse.tile as tile

    f32 = mybir.dt.float32
    bf16 = mybir.dt.bfloat16
    AF = mybir.ActivationFunctionType
    OP = mybir.AluOpType

    nc = bacc.Bacc("TRN2", target_bir_lowering=False, debug=False,
                   num_devices=NCORES)

    xt_d = nc.dram_tensor("xt", [128, KSUB * LQ], bf16,
                          kind="ExternalInput").ap()
    wqa_d = nc.dram_tensor("wqa", [128, KSUB * 512], bf16,
                           kind="ExternalInput").ap()
    wqb_d = nc.dram_tensor("wqb", [128, KSUB * 512], bf16,
                           kind="ExternalInput").ap()
    wk_d = nc.dram_tensor("wk", [128, KSUB * 256], bf16,
                          kind="ExternalInput").ap()
    wv_d = nc.dram_tensor("wv", [128, KSUB * 256], bf16,
                          kind="ExternalInput").ap()
    bqk_d = nc.dram_tensor("bqk", [128, NH + NG], f32,
                           kind="ExternalInput").ap()
    bv_d = nc.dram_tensor("bv", [1, NG * HD], f32, kind="ExternalInput").ap()
    pkt_d = nc.dram_tensor("pkt", [128, NG * PAST], bf16,
                           kind="ExternalInput").ap()
    pv_d = nc.dram_tensor("pv", [128, NG * PAST], bf16,
                          kind="ExternalInput").ap()
    rott_d = nc.dram_tensor("rott", [64, LQ], f32, kind="ExternalInput").ap()
    wo_d = nc.dram_tensor("wo", [128, NH * D], bf16,
                          kind="ExternalInput").ap()
    tri_d = nc.dram_tensor("tri", [128, 256], bf16, kind="ExternalInput").ap()
    out_d = nc.dram_tensor("out", [LQ, D], f32, kind="ExternalOutput").ap()

    scl = 1.0 / math.sqrt(HD)

    with tile.TileContext(nc) as tc:
        with (
            tc.tile_pool(name="const", bufs=1) as const,
            tc.tile_pool(name="persist", bufs=1) as persist,
        ):
            QT = persist.tile([128, NH, LQ], bf16)      # roped Q^T (perm rows)
            KT = persist.tile([128, NG, LK], bf16)      # K^T cache (perm rows)
            V = persist.tile([128, NG, KC, HD], bf16)   # [k, g, kc, hd]
            attnT = persist.tile([128, NH, LQ], bf16)   # normalized attn^T

            with (
                tc.tile_pool(name="xtp", bufs=1) as xtp,
                tc.tile_pool(name="ropec", bufs=1) as ropec,
                tc.tile_pool(name="ropew", bufs=1) as ropew,
                tc.tile_pool(name="rawp", bufs=1) as rawp,
                tc.tile_pool(name="qrawp", bufs=2) as qrawp,
            ):
                xt = xtp.tile([128, KSUB, LQ], bf16, name="xt")
                cosF = ropec.tile([128, LQ], f32)
                ssgnF = ropec.tile([128, LQ], f32)

                wq_cm = tc.tile_pool(name="wqp", bufs=1)
                wqp = wq_cm.__enter__()
                wqt = [wqp.tile([128, KSUB, 512], bf16, name=f"wq{i}")
                       for i in range(2)]
                wkv_cm = tc.tile_pool(name="wkv", bufs=1)
                wkvp = wkv_cm.__enter__()
                wkt = wkvp.tile([128, KSUB, NG * HD], bf16, name="wk")
                wvt = wkvp.tile([128, KSUB, NG * HD], bf16, name="wv")

                # ---- DMA: 3 queues in parallel (sync, gpsimd, scalar) ----
                ones_f = const.tile([128, 32], f32)
                nc.gpsimd.memset(ones_f, 1.0)
                negpi = const.tile([64, 1], f32)
                nc.gpsimd.memset(negpi, -math.pi)
                warm = const.tile([128, 512], bf16)
                nc.gpsimd.memset(warm, 0.0)

                rot_cm = tc.tile_pool(name="rotw", bufs=1)
                rotw = rot_cm.__enter__()
                rstage = rotw.tile([64, LQ], f32, name="rstage")
                s2 = rotw.tile([64, LQ], f32, name="s2")
                nc.gpsimd.dma_start(rstage, rott_d)

                # sync: even x chunks (j=0 first, the critical path)
                xt_r = xt_d.rearrange("p (ko q) -> p ko q", q=LQ)
                nc.sync.dma_start(xt[:, 0:1, :], xt_r[:, 0:1, :])
                nc.sync.dma_start(xt[:, 1:2, :], xt_r[:, 1:2, :])
                for j in range(2, 8, 2):
                    sl = slice(2 * j, 2 * (j + 1))
                    nc.sync.dma_start(xt[:, sl, :], xt_r[:, sl, :])

                # gpsimd: wk + odd x chunks + small consts
                wk_r = wk_d.rearrange("p (ko m) -> p ko m", m=NG * HD)
                sl = slice(0, 8)
                nc.gpsimd.dma_start(wkt[:, sl, :], wk_r[:, sl, :])
                for j in (1, 3):
                    sl = slice(2 * j, 2 * (j + 1))
                    nc.gpsimd.dma_start(xt[:, sl, :], xt_r[:, sl, :])
                sl = slice(8, 16)
                nc.gpsimd.dma_start(wkt[:, sl, :], wk_r[:, sl, :])
                for j in (5, 7):
                    sl = slice(2 * j, 2 * (j + 1))
                    nc.gpsimd.dma_start(xt[:, sl, :], xt_r[:, sl, :])
                bias_qk = const.tile([128, NH + NG], f32)
                nc.gpsimd.dma_start(bias_qk, bqk_d)
                bv_sb = const.tile([1, NG * HD], f32)
                nc.gpsimd.dma_start(bv_sb, bv_d)
                tri2 = const.tile([128, 2, 128], bf16)
                nc.gpsimd.dma_start(tri2,
                                    tri_d.rearrange("p (i f) -> p i f", f=128))
                ones_c = const.tile([128, 32], bf16)
                nc.vector.tensor_copy(ones_c, ones_f)

                # scalar queue: first wq chunk, trig setup, then the
                # non-critical weights/caches trailing the first wave
                wqa_r = wqa_d.rearrange("p (ko m) -> p ko m", m=512)
                wqb_r = wqb_d.rearrange("p (ko m) -> p ko m", m=512)
                wv_r = wv_d.rearrange("p (ko m) -> p ko m", m=NG * HD)
                sl0 = slice(0, 4)
                nc.scalar.dma_start(wqt[0][:, sl0, :], wqa_r[:, sl0, :])

                # rotary tables: rows 0:64 = even dims, 64:128 = odd;
                # ssgnF = -sin on top, +sin on bottom, so
                # roped = src*cosF + swap(src)*ssgnF.
                # -sin(x) = sin(x - pi); cos(x) = 1 - 2*sin^2(x/2)
                nc.scalar.activation(ssgnF[0:64], rstage, AF.Sin,
                                     bias=negpi)
                nc.scalar.activation(s2, rstage, AF.Sin, scale=0.5)
                nc.vector.tensor_mul(s2, s2, s2)
                nc.vector.tensor_scalar(cosF[0:64], s2, -2.0, 1.0,
                                        OP.mult, OP.add)
                nc.vector.tensor_scalar_mul(s2, ssgnF[0:64], -1.0)
                nc.sync.dma_start(ssgnF[64:128], s2)
                nc.sync.dma_start(cosF[64:128], cosF[0:64])
                rot_cm.__exit__(None, None, None)

                for c in range(1, 4):
                    sl = slice(4 * c, 4 * (c + 1))
                    nc.scalar.dma_start(wqt[0][:, sl, :], wqa_r[:, sl, :])
                nc.scalar.dma_start(wvt, wv_r)
                nc.scalar.dma_start(wqt[1], wqb_r)
                nc.scalar.dma_start(
                    KT[:, :, 0:PAST],
                    pkt_d.rearrange("p (g f) -> p g f", g=NG))
                nc.scalar.dma_start(
                    V[:, :, 0:PAST // 128, :],
                    pv_d.rearrange("p (g kc hd) -> p g kc hd", g=NG, hd=HD))

                bv_rep = const.tile([128, NG * HD], f32)
                nc.gpsimd.partition_broadcast(bv_rep, bv_sb)

                def rope(src, dst):
                    # src [128, LQ] f32 (clobbered); dst any dtype
                    swp = ropew.tile([128, LQ], f32, tag="swp")
                    nc.sync.dma_start(swp[0:64], src[64:128])
                    nc.sync.dma_start(swp[64:128], src[0:64])
                    t = ropew.tile([128, LQ], f32, tag="ropet")
                    nc.vector.tensor_mul(t, swp, ssgnF)
                    nc.vector.tensor_mul(src, src, cosF)
                    nc.vector.tensor_tensor(dst, src, t, OP.add)

                # ---- PE p-state warm-up during the initial DMA wait ----
                with tc.tile_pool(name="pswm", bufs=1, space="PSUM") as pswm:
                    wm = pswm.tile([128, 512], f32)
                    for _ in range(5):
                        nc.tensor.matmul(wm[0:32, :], ones_c, warm,
                                         start=True, stop=True,
                                         skip_group_check=True)

                # ---- projections: one psum pool, slice-reused, so no
                # pool barrier ever parks the PE between phases ----
                kraws = [rawp.tile([128, LQ], f32, name=f"kraw{g}")
                         for g in range(NG)]
                qraw01 = [rawp.tile([128, LQ], f32, name=f"qraw{hl}")
                          for hl in range(2)]
                with tc.tile_pool(name="pskp", bufs=1, space="PSUM") as pskp:
                    kps = [pskp.tile([128, 512], f32, name=f"kps{i}")
                           for i in range(4)]
                    qps = [pskp.tile([128, 512], f32, name=f"qps{i}")
                           for i in range(4)]
                    # K + Q(h0,h1), chunk-interleaved with the xt DMA
                    for j in range(8):            # 2-ko xt chunks
                        for g in range(NG):
                            for qc in range(QC):
                                for kk in range(2):
                                    ko = 2 * j + kk
                                    nc.tensor.matmul(
                                        kps[g * QC + qc],
                                        wkt[:, ko,
                                            g * HD:(g + 1) * HD],
                                        xt[:, ko,
                                           qc * 512:(qc + 1) * 512],
                                        start=(ko == 0),
                                        stop=(ko == KSUB - 1),
                                        skip_group_check=True)
                        for hl in range(2):
                            for qc in range(QC):
                                for kk in range(2):
                                    ko = 2 * j + kk
                                    nc.tensor.matmul(
                                        qps[hl * QC + qc],
                                        wqt[0][:, ko,
                                               hl * 128:(hl + 1) * 128],
                                        xt[:, ko,
                                           qc * 512:(qc + 1) * 512],
                                        start=(ko == 0),
                                        stop=(ko == KSUB - 1),
                                        skip_group_check=True)
                    # drain all 8 psum banks quickly (bias adds only; the
                    # slow ropes read the raw SBUF tiles, not psum)
                    for g in range(NG):
                        for qc in range(QC):
                            nc.vector.tensor_scalar_add(
                                kraws[g][:, qc * 512:(qc + 1) * 512],
                                kps[g * QC + qc],
                                bias_qk[:, NH + g:NH + g + 1])
                    for hl in range(2):
                        for qc in range(QC):
                            nc.vector.tensor_scalar_add(
                                qraw01[hl][:, qc * 512:(qc + 1) * 512],
                                qps[hl * QC + qc],
                                bias_qk[:, hl:hl + 1])

                    # rope what attention g0 needs first
                    rope(kraws[0], KT[:, 0, PAST:])
                    rope(qraw01[0], QT[:, 0, :])
                    rope(qraw01[1], QT[:, 1, :])

                    # V proj into slices of the freed K/Q banks
                    for qs in range(QS):
                        bank = (kps + qps)[qs]
                        ps = bank[:, 0:NG * HD]
                        for ko in range(KSUB):
                            nc.tensor.matmul(
                                ps,
                                xt[:, ko, qs * 128:(qs + 1) * 128],
                                wvt[:, ko, :],
                                start=(ko == 0),
                                stop=(ko == KSUB - 1),
                                skip_group_check=True)
                        for g in range(NG):
                            nc.vector.scalar_tensor_tensor(
                                V[:, g, PAST // 128 + qs, :],
                                ps[:, g * HD:(g + 1) * HD], 1.0,
                                bv_rep[:, g * HD:(g + 1) * HD],
                                OP.mult, OP.add)

                    rope(kraws[1], KT[:, 1, PAST:])

                    # Q proj h2..h7 on the same banks
                    def q_proj(hh, hls, banks):
                        for n, hl in enumerate(hls):
                            h = hh * 4 + hl
                            qraw = qrawp.tile([128, LQ], f32, tag="qraw")
                            msl = slice(hl * 128, (hl + 1) * 128)
                            for qc in range(QC):
                                qsl = slice(qc * 512, (qc + 1) * 512)
                                ps = banks[(2 * n + qc) % len(banks)]
                                for ko in range(KSUB):
                                    nc.tensor.matmul(
                                        ps, wqt[hh][:, ko, msl],
                                        xt[:, ko, qsl],
                                        start=(ko == 0),
                                        stop=(ko == KSUB - 1),
                                        skip_group_check=True)
                                nc.vector.tensor_scalar_add(
                                    qraw[:, qsl], ps, bias_qk[:, h:h + 1])
                            rope(qraw, QT[:, h, :])

                    q_proj(0, [2, 3], qps)
                    q_proj(1, [0, 1, 2, 3], kps)

                wkv_cm.__exit__(None, None, None)
                wq_cm.__exit__(None, None, None)

                # ---- attention + output projection ----
                with tc.tile_pool(name="wop", bufs=1) as wop:
                    wot = wop.tile([128, NH, D], bf16, name="wo")
                    wo_r = wo_d.rearrange("p (h n) -> p h n", n=D)
                    nc.gpsimd.dma_start(wot[:, 0:4, :], wo_r[:, 0:4, :])
                    nc.gpsimd.dma_start(wot[:, 4:8, :], wo_r[:, 4:8, :])

                    with (
                        tc.tile_pool(name="ptp", bufs=6) as ptp,
                        tc.tile_pool(name="unp", bufs=4) as unp,
                        tc.tile_pool(name="recp", bufs=2) as recp,
                        tc.tile_pool(name="psst", bufs=2,
                                     space="PSUM") as psst,
                        tc.tile_pool(name="pspv", bufs=2,
                                     space="PSUM") as pspv,
                        tc.tile_pool(name="psdn", bufs=2,
                                     space="PSUM") as psdn,
                    ):
                        def attn_group(g):
                            for qc in range(QC):
                                act = active[qc]
                                kc0, kcL = act[0][0], act[-1][0]
                                nact = len(act)
                                pidx = {0: [], 1: []}
                                for idx in range(nact):
                                    pidx[idx % 2].append(idx)
                                pfirst = {p: v[0] for p, v in pidx.items()}
                                plast = {p: v[-1] for p, v in pidx.items()}
                                assert act[pfirst[0]][1] == 0
                                assert act[pfirst[1]][1] == 0
                                for half in range(2):
                                    h0 = g * GS + half * 2
                                    ps_pv = [
                                        pspv.tile([128, 512], f32, tag="pv",
                                                  name=f"pv{half}{hi}")
                                        for hi in range(2)]
                                    ps_dn = psdn.tile([128, 512], f32,
                                                      tag="dn")
                                    pend = []

                                    def pv_den(pair):
                                        # PV for both kc of the pair, then
                                        # the four M=32 den matmuls
                                        # back-to-back: head x kc-parity on
                                        # PE columns 0/32/64/96 overlap
                                        for (kc, dcol, pt), _ in pair:
                                            for i in range(2):
                                                nc.tensor.matmul(
                                                    ps_pv[i][:, dcol:512],
                                                    V[:, g, kc, :],
                                                    pt[:, i, dcol:512],
                                                    start=(kc == kc0),
                                                    stop=(kc == kcL),
                                                    skip_group_check=True)
                                        for (kc, dcol, pt), idx in pair:
                                            par = idx % 2
                                            for i in range(2):
                                                row = 64 * par + 32 * i
                                                nc.tensor.matmul(
                                                    ps_dn[row:row + 32,
                                                          dcol:512],
                                                    ones_c,
                                                    pt[:, i, dcol:512],
                                                    start=(idx
                                                           == pfirst[par]),
                                                    stop=(idx
                                                          == plast[par]),
                                                    tile_position=(0, row),
                                                    skip_group_check=True)

                                    for idx, (kc, dcol, diag) in \
                                            enumerate(act):
                                        st = psst.tile([128, 2, 512], f32,
                                                       tag="st")
                                        for i in range(2):
                                            nc.tensor.matmul(
                                                st[:, i, dcol:512],
                                                KT[:, g,
                                                   kc * 128:(kc + 1) * 128],
                                                QT[:, h0 + i,
                                                   qc * 512 + dcol:
                                                   (qc + 1) * 512],
                                                start=True, stop=True)
                                        if len(pend) == 4:
                                            pv_den(pend[0:2])
                                            del pend[0:2]
                                        pt = ptp.tile([128, 2, 512], bf16,
                                                      tag="pt")
                                        nc.scalar.activation(
                                            pt[:, :, dcol:512],
                                            st[:, :, dcol:512],
                                            AF.Exp, scale=scl)
                                        if diag:
                                            for i in range(2):
                                                nc.vector.tensor_mul(
                                                    pt[:, i,
                                                       dcol:dcol + 128],
                                                    pt[:, i,
                                                       dcol:dcol + 128],
                                                    tri2[:, i, :])
                                        pend.append(((kc, dcol, pt), idx))
                                    while pend:
                                        pv_den(pend[0:2])
                                        del pend[0:2]

                                    # tail (DVE + gpsimd only, so the ACT
                                    # engine never delays the next exps):
                                    # drain PV, reassemble den = even+odd,
                                    # one bf16 reciprocal for both heads
                                    uns = []
                                    for i in range(2):
                                        un = unp.tile([128, 512], f32,
                                                      tag="un")
                                        nc.vector.tensor_copy(un, ps_pv[i])
                                        uns.append(un)
                                    sb_dn = recp.tile([128, 512], f32,
                                                      tag="sbdn")
                                    nc.vector.tensor_copy(sb_dn, ps_dn)
                                    dnA = recp.tile([2, 512], f32,
                                                    tag="dnA")
                                    dnB = recp.tile([2, 512], f32,
                                                    tag="dnB")
                                    nc.sync.dma_start(dnA[0:1],
                                                      sb_dn[0:1, :])
                                    nc.sync.dma_start(dnA[1:2],
                                                      sb_dn[32:33, :])
                                    nc.sync.dma_start(dnB[0:1],
                                                      sb_dn[64:65, :])
                                    nc.sync.dma_start(dnB[1:2],
                                                      sb_dn[96:97, :])
                                    dnS = recp.tile([2, 512], bf16,
                                                    tag="dnS")
                                    rec = recp.tile([2, 512], bf16,
                                                    tag="rec")
                                    with nc.allow_low_precision(
                                            reason="den in bf16: 0.4% rel "
                                                   "err, inside the 2e-2 "
                                                   "gate; 2x DVE rate"):
                                        nc.vector.tensor_tensor(
                                            dnS, dnA, dnB, OP.add)
                                        nc.vector.reciprocal(rec, dnS)
                                    rec1 = recp.tile([1, 512], bf16,
                                                     tag="rec1")
                                    nc.sync.dma_start(rec1, rec[1:2, :])
                                    qsl = slice(qc * 512, (qc + 1) * 512)
                                    for i in range(2):
                                        r128 = unp.tile([128, 512], bf16,
                                                        tag="r128")
                                        nc.gpsimd.partition_broadcast(
                                            r128,
                                            rec[0:1, :] if i == 0 else rec1)
                                        nc.vector.tensor_mul(
                                            attnT[:, h0 + i, qsl],
                                            uns[i], r128)

                        attn_group(0)
                        attn_group(1)

                    # ---- output projection ----
                    with (
                        tc.tile_pool(name="otp", bufs=3) as otp,
                        tc.tile_pool(name="pso", bufs=3,
                                     space="PSUM") as pso,
                    ):
                        for qs in range(QS):
                            asl = slice(qs * 128, (qs + 1) * 128)
                            for nh in range(2):
                                ps = pso.tile([128, 1024], f32, tag="o")
                                for h in range(NH):
                                    for nn in range(2):
                                        ncH = nh * 2 + nn
                                        nc.tensor.matmul(
                                            ps[:, nn * 512:(nn + 1) * 512],
                                            attnT[:, h, asl],
                                            wot[:, h,
                                                ncH * 512:(ncH + 1) * 512],
                                            start=(h == 0),
                                            stop=(h == NH - 1),
                                            skip_group_check=True)
                                for nn in range(2):
                                    ncH = nh * 2 + nn
                                    ot = otp.tile([128, 512], f32,
                                                  tag="ot")
                                    nc.scalar.activation(
                                        ot, ps[:, nn * 512:(nn + 1) * 512],
                                        AF.Copy)
                                    qeng = nc.sync if nn == 0 \
                                        else nc.gpsimd
                                    qeng.dma_start(
                                        out_d[qs * 128:(qs + 1) * 128,
                                              ncH * 512:(ncH + 1) * 512],
                                        ot)

    nc.compile()
    return nc


def _classify_mask(mask):
    """Per-[512q x 128k] tile -> active[qc] = [(kc, dcol, diag)].

    Verifies the mask is the causal+past pattern the kernel assumes:
    full tiles, skip tiles, and diagonal tiles of the form
    [masked rows | triangular block | allowed rows] split at dcol.
    """
    m = np.asarray(mask)
    tril = np.tril(np.ones((128, 128), bool))  # [q, k]: allow k <= q
    active = {}
    for qc in range(QC):
        lst = []
        for kc in range(KC):
            t = m[qc * 512:(qc + 1) * 512, kc * 128:(kc + 1) * 128]  # [q, k]
            if t.all():
                lst.append((kc, 0, False))
            elif not t.any():
                continue
            else:
                rows_any = np.nonzero(t.any(axis=1))[0]
                dcol = int(rows_any[0])
                assert dcol % 128 == 0, f"unexpected mask tile ({qc},{kc})"
                assert (t[dcol:dcol + 128] == tril).all(), \
                    f"non-causal tile ({qc},{kc})"
                assert t[dcol + 128:].all() or dcol + 128 >= 512
                assert not t[:dcol].any()
                lst.append((kc, dcol, True))
        assert lst and lst[0][1] == 0 and not lst[0][2], "first tile not full"
        assert len(lst) >= 2 and lst[1][1] == 0, "second tile not full"
        active[qc] = lst
    return active


def _prep_in_maps(inputs):
    import ml_dtypes
    c32 = lambda a: np.ascontiguousarray(a, dtype=np.float32)
    c16 = lambda a: np.ascontiguousarray(a, dtype=ml_dtypes.bfloat16)
    x = np.asarray(inputs["x"], np.float32)
    rot = np.asarray(inputs["rotary_freqs"], np.float32)
    pk = np.asarray(inputs["past_k"], np.float32)
    pv = np.asarray(inputs["past_v"], np.float32)
    Wq = np.asarray(inputs["Wq"], np.float32)
    bq = np.asarray(inputs["bq"], np.float32)
    Wk = np.asarray(inputs["Wk"], np.float32)
    bk = np.asarray(inputs["bk"], np.float32)
    Wv = np.asarray(inputs["Wv"], np.float32)
    bv = np.asarray(inputs["bv"], np.float32)
    Wo = np.asarray(inputs["Wo"], np.float32)

    tri = np.triu(np.ones((128, 128), np.float32))  # [k, q]: allow k <= q
    tri2 = np.concatenate([tri, tri], axis=1)

    def tilize(w):
        # [K, M] -> [128, (K//128) * M], partition-contiguous runs
        K, M = w.shape
        return np.ascontiguousarray(
            w.reshape(K // 128, 128, M).transpose(1, 0, 2).reshape(128, -1))

    in_maps = []
    for c in range(NCORES):
        b, half = c // 2, c % 2
        h0 = half * NH
        g0 = half * NG
        qcols = np.concatenate(
            [Wq[:, (h0 + h) * HD + _PERM] for h in range(NH)], axis=1)
        kcols = np.concatenate(
            [Wk[:, (g0 + g) * HD + _PERM] for g in range(NG)], axis=1)
        bqk = np.stack(
            [bq[(h0 + h) * HD + _PERM] for h in range(NH)]
            + [bk[(g0 + g) * HD + _PERM] for g in range(NG)], axis=1)
        pkt = np.stack([pk[b, g0 + g][:, _PERM].T for g in range(NG)],
                       axis=1)                       # [128, NG, PAST]
        pvt = pv[b, g0:g0 + NG].reshape(NG, PAST // 128, 128, HD) \
            .transpose(2, 0, 1, 3)                   # [128, NG, kc, HD]
        wo = Wo[h0 * HD:(h0 + NH) * HD, :].reshape(NH, HD, D) \
            .transpose(1, 0, 2)                      # [128, NH, D]
        m = {
            "xt": c16(tilize(x[b].T)),
            "wqa": c16(tilize(qcols[:, 0:512])),
            "wqb": c16(tilize(qcols[:, 512:1024])),
            "wk": c16(tilize(kcols)),
            "wv": c16(tilize(Wv[:, g0 * HD:(g0 + NG) * HD])),
            "bqk": c32(bqk),
            "bv": c32(bv[g0 * HD:(g0 + NG) * HD][None, :]),
            "pkt": c16(pkt.reshape(128, -1)),
            "pv": c16(pvt.reshape(128, -1)),
            "rott": c32(rot.T),
            "wo": c16(wo.reshape(128, -1)),
            "tri": c16(tri2),
        }
        in_maps.append(m)
    return in_maps


def _run(inputs, trace=False):
    from concourse import bass_utils

    active = _classify_mask(inputs["mask"])
    key = tuple(sorted((qc, tuple(v)) for qc, v in active.items()))
    if key not in _PROG_CACHE:
        _PROG_CACHE[key] = _build_program(active)
    nc = _PROG_CACHE[key]

    in_maps = _prep_in_maps(inputs)
    res = bass_utils.run_bass_kernel_spmd(
        nc, in_maps, list(range(NCORES)), trace=trace,
        trace_cores=list(range(NCORES)) if trace else None)

    bo = np.asarray(inputs["bo"], np.float32)
    out = np.empty((B, LQ, D), np.float32)
    for b in range(B):
        out[b] = res.results[2 * b]["out"] + res.results[2 * b + 1]["out"] \
            + bo[None, :]
    return out, res


def kernel(**inputs) -> np.ndarray:
    out, _ = _run(inputs, trace=False)
    return out


# revision 17
# speedup vs baseline: 1.0060x; 1.0060x over previous
"""Cached grouped-query multi-head attention on 8 Trainium2 cores.

Sharding: core c -> batch b = c//2, head-half = c%2 (8 of 16 heads, 2 of 4
KV groups per core). Wq/Wk column-parallel, Wo row-parallel; the two
partial Wo products per batch are summed on the host (the "all-reduce"),
which also adds bo.

All matmuls are bf16 (the PE moving-data port is 2 B/cycle/partition, so
fp8 DoubleRow only matches bf16 per contraction pair and any residual
compensation makes it slower; bare fp8 fails the 2e-2 gate).

Schedule notes:
- Inputs stream on three DMA queues; the rings share 16 DMA engines and
  saturate ~390 GB/s, so only the critical tensors (xt, wqa, wk) go in
  the first wave; wv/wqb/pkt/pv trail, wo is prefetched mid-attention.
- A short PE warm-up chain rides the initial DMA wait so the clock is at
  full p-state when real work lands.
- All projections share one psum pool by slicing the kps/qps tiles, so
  no pool-open barrier ever idles the PE between projection phases; both
  Q-proj halves run before attention (their ropes overlap attention PE).
- Attention per (g, qc, head-pair): per-head score matmuls into a
  [128,2,512] psum supertile -> one exp for both heads -> per-head PV.
  pv/den are emitted per kc-PAIR so the four M=32 denominator matmuls
  (head x kc-parity at PE columns 0/32/64/96) run concurrently, costing
  ~0.5 matmuls per key tile. The tail runs entirely on DVE+gpsimd (the
  ACT engine only ever runs exp, so scores never wait on it): psum
  drains, den = even+odd, one bf16 reciprocal (2x DVE rate) per pair of
  heads, and the normalize multiplies.
"""

import math
import sys

import numpy as np

sys.path.insert(0, "/opt/trn_rl_repo")

B, LQ, D = 4, 1024, 2048
H, G = 16, 4
HD = 128            # head dim
GS = H // G         # heads per group
PAST = 1024
LK = PAST + LQ      # 2048
NCORES = 8
NH = 8              # local heads per core
NG = 2              # local groups per core
KSUB = D // 128     # 16 contraction subtiles over D
QC = LQ // 512      # 2 query chunks of 512
QS = LQ // 128      # 8 query subtiles of 128
KC = LK // 128      # 16 key chunks of 128

_PERM = np.concatenate([np.arange(0, HD, 2), np.arange(1, HD, 2)])
_PROG_CACHE = {}


def _build_program(active):
    """active[qc] = [(kc, dcol, diag)]: dcol = first allowed query column
    (0 for full tiles); diag tiles have a triangular [128,128] block at
    query columns [dcol, dcol+128) and are fully allowed after it."""
    import concourse.bacc as bacc
    import concourse.mybir as mybir
    import concourse.tile as tile

    f32 = mybir.dt.float32
    bf16 = mybir.dt.bfloat16
    AF = mybir.ActivationFunctionType
    OP = mybir.AluOpType

    nc = bacc.Bacc("TRN2", target_bir_lowering=False, debug=False,
                   num_devices=NCORES)

    xt_d = nc.dram_tensor("xt", [128, KSUB * LQ], bf16,
                          kind="ExternalInput").ap()
    wqa_d = nc.dram_tensor("wqa", [128, KSUB * 512], bf16,
                           kind="ExternalInput").ap()
    wqb_d = nc.dram_tensor("wqb", [128, KSUB * 512], bf16,
                           kind="ExternalInput").ap()
    wk_d = nc.dram_tensor("wk", [128, KSUB * 256], bf16,
                          kind="ExternalInput").ap()
    wv_d = nc.dram_tensor("wv", [128, KSUB * 256], bf16,
                          kind="ExternalInput").ap()
    bqk_d = nc.dram_tensor("bqk", [128, NH + NG], f32,
                           kind="ExternalInput").ap()
    bv_d = nc.dram_tensor("bv", [1, NG * HD], f32, kind="ExternalInput").ap()
    pkt_d = nc.dram_tensor("pkt", [128, NG * PAST], bf16,
                           kind="ExternalInput").ap()
    pv_d = nc.dram_tensor("pv", [128, NG * PAST], bf16,
                          kind="ExternalInput").ap()
    rott_d = nc.dram_tensor("rott", [64, LQ], f32, kind="ExternalInput").ap()
    wo_d = nc.dram_tensor("wo", [128, NH * D], bf16,
                          kind="ExternalInput").ap()
    tri_d = nc.dram_tensor("tri", [128, 256], bf16, kind="ExternalInput").ap()
    out_d = nc.dram_tensor("out", [LQ, D], f32, kind="ExternalOutput").ap()

    scl = 1.0 / math.sqrt(HD)

    with tile.TileContext(nc) as tc:
        with (
            tc.tile_pool(name="const", bufs=1) as const,
            tc.tile_pool(name="persist", bufs=1) as persist,
        ):
            QT = persist.tile([128, NH, LQ], bf16)      # roped Q^T (perm rows)
            KT = persist.tile([128, NG, LK], bf16)      # K^T cache (perm rows)
            V = persist.tile([128, NG, KC, HD], bf16)   # [k, g, kc, hd]
            attnT = persist.tile([128, NH, LQ], bf16)   # normalized attn^T

            with (
                tc.tile_pool(name="xtp", bufs=1) as xtp,
                tc.tile_pool(name="ropec", bufs=1) as ropec,
                tc.tile_pool(name="ropew", bufs=1) as ropew,
                tc.tile_pool(name="rawp", bufs=1) as rawp,
                tc.tile_pool(name="qrawp", bufs=4) as qrawp,
            ):
                xt = xtp.tile([128, KSUB, LQ], bf16, name="xt")
                cosF = ropec.tile([128, LQ], f32)
                ssgnF = ropec.tile([128, LQ], f32)

                wq_cm = tc.tile_pool(name="wqp", bufs=1)
                wqp = wq_cm.__enter__()
                wqt = [wqp.tile([128, KSUB, 512], bf16, name=f"wq{i}")
                       for i in range(2)]
                wkv_cm = tc.tile_pool(name="wkv", bufs=1)
                wkvp = wkv_cm.__enter__()
                wkt = wkvp.tile([128, KSUB, NG * HD], bf16, name="wk")
                wvt = wkvp.tile([128, KSUB, NG * HD], bf16, name="wv")

                # ---- DMA: 3 queues in parallel (sync, gpsimd, scalar) ----
                ones_f = const.tile([128, 32], f32)
                nc.gpsimd.memset(ones_f, 1.0)
                negpi = const.tile([64, 1], f32)
                nc.gpsimd.memset(negpi, -math.pi)
                warm = const.tile([128, 512], bf16)
                nc.gpsimd.memset(warm, 0.0)

                rot_cm = tc.tile_pool(name="rotw", bufs=1)
                rotw = rot_cm.__enter__()
                rstage = rotw.tile([64, LQ], f32, name="rstage")
                s2 = rotw.tile([64, LQ], f32, name="s2")
                nc.gpsimd.dma_start(rstage, rott_d)

                # sync: even x chunks (j=0 first, the critical path)
                xt_r = xt_d.rearrange("p (ko q) -> p ko q", q=LQ)
                nc.sync.dma_start(xt[:, 0:1, :], xt_r[:, 0:1, :])
                nc.sync.dma_start(xt[:, 1:2, :], xt_r[:, 1:2, :])
                for j in range(2, 8, 2):
                    sl = slice(2 * j, 2 * (j + 1))
                    nc.sync.dma_start(xt[:, sl, :], xt_r[:, sl, :])

                # gpsimd: wk + odd x chunks + small consts
                wk_r = wk_d.rearrange("p (ko m) -> p ko m", m=NG * HD)
                sl = slice(0, 8)
                nc.gpsimd.dma_start(wkt[:, sl, :], wk_r[:, sl, :])
                for j in (1, 3):
                    sl = slice(2 * j, 2 * (j + 1))
                    nc.gpsimd.dma_start(xt[:, sl, :], xt_r[:, sl, :])
                sl = slice(8, 16)
                nc.gpsimd.dma_start(wkt[:, sl, :], wk_r[:, sl, :])
                for j in (5, 7):
                    sl = slice(2 * j, 2 * (j + 1))
                    nc.gpsimd.dma_start(xt[:, sl, :], xt_r[:, sl, :])
                bias_qk = const.tile([128, NH + NG], f32)
                nc.gpsimd.dma_start(bias_qk, bqk_d)
                bv_sb = const.tile([1, NG * HD], f32)
                nc.gpsimd.dma_start(bv_sb, bv_d)
                tri2 = const.tile([128, 2, 128], bf16)
                nc.gpsimd.dma_start(tri2,
                                    tri_d.rearrange("p (i f) -> p i f", f=128))
                ones_c = const.tile([128, 32], bf16)
                nc.vector.tensor_copy(ones_c, ones_f)

                # scalar queue: first wq chunk, trig setup, then the
                # non-critical weights/caches trailing the first wave
                wqa_r = wqa_d.rearrange("p (ko m) -> p ko m", m=512)
                wqb_r = wqb_d.rearrange("p (ko m) -> p ko m", m=512)
                wv_r = wv_d.rearrange("p (ko m) -> p ko m", m=NG * HD)
                sl0 = slice(0, 4)
                nc.scalar.dma_start(wqt[0][:, sl0, :], wqa_r[:, sl0, :])

                # rotary tables: rows 0:64 = even dims, 64:128 = odd;
                # ssgnF = -sin on top, +sin on bottom, so
                # roped = src*cosF + swap(src)*ssgnF.
                # -sin(x) = sin(x - pi); cos(x) = 1 - 2*sin^2(x/2)
                nc.scalar.activation(ssgnF[0:64], rstage, AF.Sin,
                                     bias=negpi)
                nc.scalar.activation(s2, rstage, AF.Sin, scale=0.5)
                nc.vector.tensor_mul(s2, s2, s2)
                nc.vector.tensor_scalar(cosF[0:64], s2, -2.0, 1.0,
                                        OP.mult, OP.add)
                nc.vector.tensor_scalar_mul(s2, ssgnF[0:64], -1.0)
                nc.sync.dma_start(ssgnF[64:128], s2)
                nc.sync.dma_start(cosF[64:128], cosF[0:64])
                rot_cm.__exit__(None, None, None)

                for c in range(1, 4):
                    sl = slice(4 * c, 4 * (c + 1))
                    nc.scalar.dma_start(wqt[0][:, sl, :], wqa_r[:, sl, :])
                nc.scalar.dma_start(wvt, wv_r)
                nc.scalar.dma_start(wqt[1], wqb_r)
                nc.scalar.dma_start(
                    KT[:, :, 0:PAST],
                    pkt_d.rearrange("p (g f) -> p g f", g=NG))
                nc.scalar.dma_start(
                    V[:, :, 0:PAST // 128, :],
                    pv_d.rearrange("p (g kc hd) -> p g kc hd", g=NG, hd=HD))

                bv_rep = const.tile([128, NG * HD], f32)
                nc.gpsimd.partition_broadcast(bv_rep, bv_sb)

                def rope(src, dst):
                    # src [128, LQ] f32 (clobbered); dst any dtype
                    swp = ropew.tile([128, LQ], f32, tag="swp")
                    nc.sync.dma_start(swp[0:64], src[64:128])
                    nc.sync.dma_start(swp[64:128], src[0:64])
                    t = ropew.tile([128, LQ], f32, tag="ropet")
                    nc.vector.tensor_mul(t, swp, ssgnF)
                    nc.vector.tensor_mul(src, src, cosF)
                    nc.vector.tensor_tensor(dst, src, t, OP.add)

                # ---- PE p-state warm-up during the initial DMA wait ----
                with tc.tile_pool(name="pswm", bufs=1, space="PSUM") as pswm:
                    wm = pswm.tile([128, 512], f32)
                    for _ in range(5):
                        nc.tensor.matmul(wm[0:32, :], ones_c, warm,
                                         start=True, stop=True,
                                         skip_group_check=True)

                # ---- projections: one psum pool, slice-reused, so no
                # pool barrier ever parks the PE between phases ----
                kraws = [rawp.tile([128, LQ], f32, name=f"kraw{g}")
                         for g in range(NG)]
                qraw01 = [rawp.tile([128, LQ], f32, name=f"qraw{hl}")
                          for hl in range(2)]
                with tc.tile_pool(name="pskp", bufs=1, space="PSUM") as pskp:
                    kps = [pskp.tile([128, 512], f32, name=f"kps{i}")
                           for i in range(4)]
                    qps = [pskp.tile([128, 512], f32, name=f"qps{i}")
                           for i in range(4)]
                    # K + Q(h0,h1), chunk-interleaved with the xt DMA
                    for j in range(8):            # 2-ko xt chunks
                        for g in range(NG):
                            for qc in range(QC):
                                for kk in range(2):
                                    ko = 2 * j + kk
                                    nc.tensor.matmul(
                                        kps[g * QC + qc],
                                        wkt[:, ko,
                                            g * HD:(g + 1) * HD],
                                        xt[:, ko,
                                           qc * 512:(qc + 1) * 512],
                                        start=(ko == 0),
                                        stop=(ko == KSUB - 1),
                                        skip_group_check=True)
                        for hl in range(2):
                            for qc in range(QC):
                                for kk in range(2):
                                    ko = 2 * j + kk
                                    nc.tensor.matmul(
                                        qps[hl * QC + qc],
                                        wqt[0][:, ko,
                                               hl * 128:(hl + 1) * 128],
                                        xt[:, ko,
                                           qc * 512:(qc + 1) * 512],
                                        start=(ko == 0),
                                        stop=(ko == KSUB - 1),
                                        skip_group_check=True)
                    # drain all 8 psum banks quickly (bias adds only; the
                    # slow ropes read the raw SBUF tiles, not psum)
                    for g in range(NG):
                        for qc in range(QC):
                            nc.vector.tensor_scalar_add(
                                kraws[g][:, qc * 512:(qc + 1) * 512],
                                kps[g * QC + qc],
                                bias_qk[:, NH + g:NH + g + 1])
                    for hl in range(2):
                        for qc in range(QC):
                            nc.vector.tensor_scalar_add(
                                qraw01[hl][:, qc * 512:(qc + 1) * 512],
                                qps[hl * QC + qc],
                                bias_qk[:, hl:hl + 1])

                    # rope what attention g0 needs first
                    rope(kraws[0], KT[:, 0, PAST:])
                    rope(qraw01[0], QT[:, 0, :])
                    rope(qraw01[1], QT[:, 1, :])

                    # V proj into slices of the freed K/Q banks
                    for qs in range(QS):
                        bank = (kps + qps)[qs]
                        ps = bank[:, 0:NG * HD]
                        for ko in range(KSUB):
                            nc.tensor.matmul(
                                ps,
                                xt[:, ko, qs * 128:(qs + 1) * 128],
                                wvt[:, ko, :],
                                start=(ko == 0),
                                stop=(ko == KSUB - 1),
                                skip_group_check=True)
                        for g in range(NG):
                            nc.vector.scalar_tensor_tensor(
                                V[:, g, PAST // 128 + qs, :],
                                ps[:, g * HD:(g + 1) * HD], 1.0,
                                bv_rep[:, g * HD:(g + 1) * HD],
                                OP.mult, OP.add)

                    # Q proj h2..h7 on the same banks
                    deferred = []

                    def q_proj(hh, hls, banks, defer=False):
                        for n, hl in enumerate(hls):
                            h = hh * 4 + hl
                            qraw = qrawp.tile([128, LQ], f32, tag="qraw")
                            msl = slice(hl * 128, (hl + 1) * 128)
                            for qc in range(QC):
                                qsl = slice(qc * 512, (qc + 1) * 512)
                                ps = banks[(2 * n + qc) % len(banks)]
                                for ko in range(KSUB):
                                    nc.tensor.matmul(
                                        ps, wqt[hh][:, ko, msl],
                                        xt[:, ko, qsl],
                                        start=(ko == 0),
                                        stop=(ko == KSUB - 1),
                                        skip_group_check=True)
                                nc.vector.tensor_scalar_add(
                                    qraw[:, qsl], ps, bias_qk[:, h:h + 1])
                            if defer:
                                deferred.append((qraw, QT[:, h, :]))
                            else:
                                rope(qraw, QT[:, h, :])

                    q_proj(0, [2, 3], qps)
                    # biases only for h4..h7; their ropes (and K-g1's) are
                    # pumped into the DVE queue between attention-g0
                    # tails, so the attention pool barrier does not wait
                    # ~13us of rope work
                    q_proj(1, [0, 1, 2, 3], kps, defer=True)
                    deferred.insert(2, (kraws[1], KT[:, 1, PAST:]))

                wkv_cm.__exit__(None, None, None)
                wq_cm.__exit__(None, None, None)

                # ---- attention + output projection ----
                with tc.tile_pool(name="wop", bufs=1) as wop:
                    wot = wop.tile([128, NH, D], bf16, name="wo")
                    wo_r = wo_d.rearrange("p (h n) -> p h n", n=D)
                    nc.gpsimd.dma_start(wot[:, 0:4, :], wo_r[:, 0:4, :])
                    nc.gpsimd.dma_start(wot[:, 4:8, :], wo_r[:, 4:8, :])

                    with (
                        tc.tile_pool(name="ptp", bufs=6) as ptp,
                        tc.tile_pool(name="unp", bufs=4) as unp,
                        tc.tile_pool(name="recp", bufs=2) as recp,
                        tc.tile_pool(name="psst", bufs=2,
                                     space="PSUM") as psst,
                        tc.tile_pool(name="pspv", bufs=2,
                                     space="PSUM") as pspv,
                        tc.tile_pool(name="psdn", bufs=2,
                                     space="PSUM") as psdn,
                    ):
                        def attn_group(g, pump=False):
                            tail_i = 0
                            for qc in range(QC):
                                act = active[qc]
                                kc0, kcL = act[0][0], act[-1][0]
                                nact = len(act)
                                pidx = {0: [], 1: []}
                                for idx in range(nact):
                                    pidx[idx % 2].append(idx)
                                pfirst = {p: v[0] for p, v in pidx.items()}
                                plast = {p: v[-1] for p, v in pidx.items()}
                                assert act[pfirst[0]][1] == 0
                                assert act[pfirst[1]][1] == 0
                                for half in range(2):
                                    h0 = g * GS + half * 2
                                    ps_pv = [
                                        pspv.tile([128, 512], f32, tag="pv",
                                                  name=f"pv{half}{hi}")
                                        for hi in range(2)]
                                    ps_dn = psdn.tile([128, 512], f32,
                                                      tag="dn")
                                    pend = []

                                    def pv_den(pair):
                                        # PV for both kc of the pair, then
                                        # the four M=32 den matmuls
                                        # back-to-back: head x kc-parity on
                                        # PE columns 0/32/64/96 overlap
                                        for (kc, dcol, pt), _ in pair:
                                            for i in range(2):
                                                nc.tensor.matmul(
                                                    ps_pv[i][:, dcol:512],
                                                    V[:, g, kc, :],
                                                    pt[:, i, dcol:512],
                                                    start=(kc == kc0),
                                                    stop=(kc == kcL),
                                                    skip_group_check=True)
                                        for (kc, dcol, pt), idx in pair:
                                            par = idx % 2
                                            for i in range(2):
                                                row = 64 * par + 32 * i
                                                nc.tensor.matmul(
                                                    ps_dn[row:row + 32,
                                                          dcol:512],
                                                    ones_c,
                                                    pt[:, i, dcol:512],
                                                    start=(idx
                                                           == pfirst[par]),
                                                    stop=(idx
                                                          == plast[par]),
                                                    tile_position=(0, row),
                                                    skip_group_check=True)

                                    for idx, (kc, dcol, diag) in \
                                            enumerate(act):
                                        st = psst.tile([128, 2, 512], f32,
                                                       tag="st")
                                        for i in range(2):
                                            nc.tensor.matmul(
                                                st[:, i, dcol:512],
                                                KT[:, g,
                                                   kc * 128:(kc + 1) * 128],
                                                QT[:, h0 + i,
                                                   qc * 512 + dcol:
                                                   (qc + 1) * 512],
                                                start=True, stop=True)
                                        if len(pend) == 4:
                                            pv_den(pend[0:2])
                                            del pend[0:2]
                                        pt = ptp.tile([128, 2, 512], bf16,
                                                      tag="pt")
                                        nc.scalar.activation(
                                            pt[:, :, dcol:512],
                                            st[:, :, dcol:512],
                                            AF.Exp, scale=scl)
                                        if diag:
                                            for i in range(2):
                                                nc.vector.tensor_mul(
                                                    pt[:, i,
                                                       dcol:dcol + 128],
                                                    pt[:, i,
                                                       dcol:dcol + 128],
                                                    tri2[:, i, :])
                                        pend.append(((kc, dcol, pt), idx))
                                    while pend:
                                        pv_den(pend[0:2])
                                        del pend[0:2]

                                    # tail (DVE + gpsimd only, so the ACT
                                    # engine never delays the next exps):
                                    # drain PV, reassemble den = even+odd,
                                    # one bf16 reciprocal for both heads
                                    uns = []
                                    for i in range(2):
                                        un = unp.tile([128, 512], f32,
                                                      tag="un")
                                        nc.vector.tensor_copy(un, ps_pv[i])
                                        uns.append(un)
                                    sb_dn = recp.tile([128, 512], f32,
                                                      tag="sbdn")
                                    nc.vector.tensor_copy(sb_dn, ps_dn)
                                    dnA = recp.tile([2, 512], f32,
                                                    tag="dnA")
                                    dnB = recp.tile([2, 512], f32,
                                                    tag="dnB")
                                    nc.sync.dma_start(dnA[0:1],
                                                      sb_dn[0:1, :])
                                    nc.sync.dma_start(dnA[1:2],
                                                      sb_dn[32:33, :])
                                    nc.sync.dma_start(dnB[0:1],
                                                      sb_dn[64:65, :])
                                    nc.sync.dma_start(dnB[1:2],
                                                      sb_dn[96:97, :])
                                    dnS = recp.tile([2, 512], bf16,
                                                    tag="dnS")
                                    rec = recp.tile([2, 512], bf16,
                                                    tag="rec")
                                    with nc.allow_low_precision(
                                            reason="den in bf16: 0.4% rel "
                                                   "err, inside the 2e-2 "
                                                   "gate; 2x DVE rate"):
                                        nc.vector.tensor_tensor(
                                            dnS, dnA, dnB, OP.add)
                                        nc.vector.reciprocal(rec, dnS)
                                    rec1 = recp.tile([1, 512], bf16,
                                                     tag="rec1")
                                    nc.sync.dma_start(rec1, rec[1:2, :])
                                    qsl = slice(qc * 512, (qc + 1) * 512)
                                    for i in range(2):
                                        r128 = unp.tile([128, 512], bf16,
                                                        tag="r128")
                                        nc.gpsimd.partition_broadcast(
                                            r128,
                                            rec[0:1, :] if i == 0 else rec1)
                                        nc.vector.tensor_mul(
                                            attnT[:, h0 + i, qsl],
                                            uns[i], r128)
                                    if pump:
                                        npop = 2 if tail_i == 3 else 1
                                        for _ in range(npop):
                                            if deferred:
                                                rope(*deferred.pop(0))
                                        tail_i += 1

                        attn_group(0, pump=True)
                        assert not deferred
                        attn_group(1)

                    # ---- output projection ----
                    with (
                        tc.tile_pool(name="otp", bufs=3) as otp,
                        tc.tile_pool(name="pso", bufs=3,
                                     space="PSUM") as pso,
                    ):
                        for qs in range(QS):
                            asl = slice(qs * 128, (qs + 1) * 128)
                            for nh in range(2):
                                ps = pso.tile([128, 1024], f32, tag="o")
                                for h in range(NH):
                                    for nn in range(2):
                                        ncH = nh * 2 + nn
                                        nc.tensor.matmul(
                                            ps[:, nn * 512:(nn + 1) * 512],
                                            attnT[:, h, asl],
                                            wot[:, h,
                                                ncH * 512:(ncH + 1) * 512],
                                            start=(h == 0),
                                            stop=(h == NH - 1),
                                            skip_group_check=True)
                                for nn in range(2):
                                    ncH = nh * 2 + nn
                                    ot = otp.tile([128, 512], f32,
                                                  tag="ot")
                                    nc.scalar.activation(
                                        ot, ps[:, nn * 512:(nn + 1) * 512],
                                        AF.Copy)
                                    qeng = nc.sync if nn == 0 \
                                        else nc.gpsimd
                                    qeng.dma_start(
                                        out_d[qs * 128:(qs + 1) * 128,
                                              ncH * 512:(ncH + 1) * 512],
                                        ot)

    nc.compile()
    return nc


def _classify_mask(mask):
    """Per-[512q x 128k] tile -> active[qc] = [(kc, dcol, diag)].

    Verifies the mask is the causal+past pattern the kernel assumes:
    full tiles, skip tiles, and diagonal tiles of the form
    [masked rows | triangular block | allowed rows] split at dcol.
    """
    m = np.asarray(mask)
    tril = np.tril(np.ones((128, 128), bool))  # [q, k]: allow k <= q
    active = {}
    for qc in range(QC):
        lst = []
        for kc in range(KC):
            t = m[qc * 512:(qc + 1) * 512, kc * 128:(kc + 1) * 128]  # [q, k]
            if t.all():
                lst.append((kc, 0, False))
            elif not t.any():
                continue
            else:
                rows_any = np.nonzero(t.any(axis=1))[0]
                dcol = int(rows_any[0])
                assert dcol % 128 == 0, f"unexpected mask tile ({qc},{kc})"
                assert (t[dcol:dcol + 128] == tril).all(), \
                    f"non-causal tile ({qc},{kc})"
                assert t[dcol + 128:].all() or dcol + 128 >= 512
                assert not t[:dcol].any()
                lst.append((kc, dcol, True))
        assert lst and lst[0][1] == 0 and not lst[0][2], "first tile not full"
        assert len(lst) >= 2 and lst[1][1] == 0, "second tile not full"
        active[qc] = lst
    return active


def _prep_in_maps(inputs):
    import ml_dtypes
    c32 = lambda a: np.ascontiguousarray(a, dtype=np.float32)
    c16 = lambda a: np.ascontiguousarray(a, dtype=ml_dtypes.bfloat16)
    x = np.asarray(inputs["x"], np.float32)
    rot = np.asarray(inputs["rotary_freqs"], np.float32)
    pk = np.asarray(inputs["past_k"], np.float32)
    pv = np.asarray(inputs["past_v"], np.float32)
    Wq = np.asarray(inputs["Wq"], np.float32)
    bq = np.asarray(inputs["bq"], np.float32)
    Wk = np.asarray(inputs["Wk"], np.float32)
    bk = np.asarray(inputs["bk"], np.float32)
    Wv = np.asarray(inputs["Wv"], np.float32)
    bv = np.asarray(inputs["bv"], np.float32)
    Wo = np.asarray(inputs["Wo"], np.float32)

    tri = np.triu(np.ones((128, 128), np.float32))  # [k, q]: allow k <= q
    tri2 = np.concatenate([tri, tri], axis=1)

    def tilize(w):
        # [K, M] -> [128, (K//128) * M], partition-contiguous runs
        K, M = w.shape
        return np.ascontiguousarray(
            w.reshape(K // 128, 128, M).transpose(1, 0, 2).reshape(128, -1))

    in_maps = []
    for c in range(NCORES):
        b, half = c // 2, c % 2
        h0 = half * NH
        g0 = half * NG
        qcols = np.concatenate(
            [Wq[:, (h0 + h) * HD + _PERM] for h in range(NH)], axis=1)
        kcols = np.concatenate(
            [Wk[:, (g0 + g) * HD + _PERM] for g in range(NG)], axis=1)
        bqk = np.stack(
            [bq[(h0 + h) * HD + _PERM] for h in range(NH)]
            + [bk[(g0 + g) * HD + _PERM] for g in range(NG)], axis=1)
        pkt = np.stack([pk[b, g0 + g][:, _PERM].T for g in range(NG)],
                       axis=1)                       # [128, NG, PAST]
        pvt = pv[b, g0:g0 + NG].reshape(NG, PAST // 128, 128, HD) \
            .transpose(2, 0, 1, 3)                   # [128, NG, kc, HD]
        wo = Wo[h0 * HD:(h0 + NH) * HD, :].reshape(NH, HD, D) \
            .transpose(1, 0, 2)                      # [128, NH, D]
        m = {
            "xt": c16(tilize(x[b].T)),
            "wqa": c16(tilize(qcols[:, 0:512])),
            "wqb": c16(tilize(qcols[:, 512:1024])),
            "wk": c16(tilize(kcols)),
            "wv": c16(tilize(Wv[:, g0 * HD:(g0 + NG) * HD])),
            "bqk": c32(bqk),
            "bv": c32(bv[g0 * HD:(g0 + NG) * HD][None, :]),
            "pkt": c16(pkt.reshape(128, -1)),
            "pv": c16(pvt.reshape(128, -1)),
            "rott": c32(rot.T),
            "wo": c16(wo.reshape(128, -1)),
            "tri": c16(tri2),
        }
        in_maps.append(m)
    return in_maps


def _run(inputs, trace=False):
    from concourse import bass_utils

    active = _classify_mask(inputs["mask"])
    key = tuple(sorted((qc, tuple(v)) for qc, v in active.items()))
    if key not in _PROG_CACHE:
        _PROG_CACHE[key] = _build_program(active)
    nc = _PROG_CACHE[key]

    in_maps = _prep_in_maps(inputs)
    res = bass_utils.run_bass_kernel_spmd(
        nc, in_maps, list(range(NCORES)), trace=trace,
        trace_cores=list(range(NCORES)) if trace else None)

    bo = np.asarray(inputs["bo"], np.float32)
    out = np.empty((B, LQ, D), np.float32)
    for b in range(B):
        out[b] = res.results[2 * b]["out"] + res.results[2 * b + 1]["out"] \
            + bo[None, :]
    return out, res


def kernel(**inputs) -> np.ndarray:
    out, _ = _run(inputs, trace=False)
    return out
